# revision 20
# baseline (speedup 1.0000x reference)
"""Trainium2 Bass kernel for nn_Attention_14877766713476.

Causal multi-head attention with full-hidden RoPE:
  q,k,v = x@W{q,k,v} + b;  q,k = rope(q,k);  heads=16, hd=128;
  causal softmax attention;  out = attn@Wo + bo.

Sharding: tensor-parallel over heads across 8 cores. RoPE pairs hidden
column c with c +/- 1024, i.e. head h with head h+8 -- so core m owns
heads {m, m+8} and RoPE stays core-local. Each core computes its two
heads end-to-end and a partial output projection (rows of Wo); the host
sums the 8 partials.

All matmuls in bf16 with fp32 PSUM accumulation. Host pre-transposes
x -> xT (contraction dim on partitions) and pre-slices/casts weights,
so the device does zero transposes.

Layouts (per core, host-prepared, all bf16 unless noted):
  xT    [128, 16*4096]  col = a*4096 + t   (d = a*128 + p, t = b*2048+s)
  wq/wk/wv [128, 16*256] col = a*256 + c   (d = a*128 + p, c in 0..255)
  wo    [128, 2*2048]   col = cb*2048 + dcol  (c = cb*128 + p)
  cosT/sinT [128, 2*4096] col = cb*4096 + t   (c = cb*128 + p; sinT block0
                           negated so rope_b = q_b*cos_b + q_{1-b}*sinT_b)
  bqk   [128, 4] fp32   cols: bq cb0, bq cb1, bk cb0, bk cb1
  masks [128, 4*512]    mask_d[kj, q'] = (q' - kj >= d), d = 128*dd
  ones  [128, 128]      all ones (rowsum matmul stationary)
Output per core: out [4096, 2048] bf16 partial (this core's two heads
through Wo rows); host sums partials in fp32 and adds bv@Wo + bo.
"""

import math
from contextlib import ExitStack

import numpy as np
import ml_dtypes

N_CORES = 8
B, S, D, H = 2, 2048, 2048, 16
HD = D // H          # 128
T = B * S            # 4096
P = 128
NB = D // P          # 16 contraction blocks
NG = 4               # a-groups (DMA split granularity)
GA = NB // NG        # 4 a-blocks per group
TCH = 512            # token chunk (QKV phase free dim)
NCH = T // TCH       # 8
QBLK = 512           # query block (attention phase free dim)
NQ = S // QBLK       # 4 query blocks per (batch, head)
SCALE = 1.0 / math.sqrt(HD)

BF16 = ml_dtypes.bfloat16

_CACHE = {}
LAST_RESULTS = None


def _build_program():
    import concourse.tile as tile
    from concourse import bacc, mybir

    bf = mybir.dt.bfloat16
    f32 = mybir.dt.float32
    Act = mybir.ActivationFunctionType

    nc = bacc.Bacc("TRN2", target_bir_lowering=False, debug=False,
                   num_devices=N_CORES)

    xT = nc.dram_tensor("xT", [P, NB * T], bf, kind="ExternalInput").ap()
    wq = nc.dram_tensor("wq", [P, NB * 256], bf, kind="ExternalInput").ap()
    wk = nc.dram_tensor("wk", [P, NB * 256], bf, kind="ExternalInput").ap()
    wv = nc.dram_tensor("wv", [P, NB * 256], bf, kind="ExternalInput").ap()
    wo = nc.dram_tensor("wo", [P, 2 * D], bf, kind="ExternalInput").ap()
    cosT = nc.dram_tensor("cosT", [P, 2 * T], bf, kind="ExternalInput").ap()
    sinT = nc.dram_tensor("sinT", [P, 2 * T], bf, kind="ExternalInput").ap()
    bqk = nc.dram_tensor("bqk", [P, 4], f32, kind="ExternalInput").ap()
    masks = nc.dram_tensor("masks", [P, 4 * QBLK], bf, kind="ExternalInput").ap()
    ones = nc.dram_tensor("ones", [P, P], bf, kind="ExternalInput").ap()
    out = nc.dram_tensor("out", [T, D], bf, kind="ExternalOutput").ap()

    xT4 = xT.rearrange("p (g a t) -> p g a t", g=NG, a=GA)
    wq4 = wq.rearrange("p (g a c) -> p g a c", g=NG, a=GA)
    wk4 = wk.rearrange("p (g a c) -> p g a c", g=NG, a=GA)
    wv4 = wv.rearrange("p (g a c) -> p g a c", g=NG, a=GA)
    cosT3 = cosT.rearrange("p (c t) -> p c t", c=2)
    sinT3 = sinT.rearrange("p (c t) -> p c t", c=2)

    with tile.TileContext(nc) as tc, ExitStack() as ctx:
        const = ctx.enter_context(tc.tile_pool(name="const", bufs=1))
        persist = ctx.enter_context(tc.tile_pool(name="persist", bufs=1))
        xt_pool = ctx.enter_context(tc.tile_pool(name="xt", bufs=2))
        cs_pool = ctx.enter_context(tc.tile_pool(name="cs", bufs=2))
        raw_pool = ctx.enter_context(tc.tile_pool(name="raw", bufs=2))
        tmp_pool = ctx.enter_context(tc.tile_pool(name="tmp", bufs=4))
        exp_pool = ctx.enter_context(tc.tile_pool(name="exp", bufs=5))
        rec_pool = ctx.enter_context(tc.tile_pool(name="rec", bufs=2))
        orow_pool = ctx.enter_context(tc.tile_pool(name="orow", bufs=2))

        # weights, split into a-groups so the first matmuls start early.
        # wq group 0 goes first on the sync (HWDGE) ring, ahead of the x
        # chunks; the bulk goes on the gpsimd (SWDGE) ring, which is
        # otherwise idle, so issue serialization doesn't delay the x stream.
        wq_sb = [const.tile([P, GA, 256], bf, tag=f"wq{g}", name=f"wq_sb{g}")
                 for g in range(NG)]
        wk_sb = [const.tile([P, GA, 256], bf, tag=f"wk{g}", name=f"wk_sb{g}")
                 for g in range(NG)]
        wv_sb = [const.tile([P, GA, 256], bf, tag=f"wv{g}", name=f"wv_sb{g}")
                 for g in range(NG)]
        nc.sync.dma_start(wq_sb[0][:], wq4[:, 0])
        # chunk-0 x stream right behind wq group 0; first piece halved so
        # the very first matmul can start after ~0.5 MB of traffic
        xt0 = [xt_pool.tile([P, GA, TCH], bf, tag=f"xt{g}", name=f"xt{g}_0")
               for g in range(NG)]
        nc.sync.dma_start(xt0[0][:, 0:2, :], xT4[:, 0, 0:2, 0:TCH])
        nc.sync.dma_start(xt0[0][:, 2:4, :], xT4[:, 0, 2:4, 0:TCH])
        for g in range(1, NG):
            nc.sync.dma_start(xt0[g][:], xT4[:, g, :, 0:TCH])
        for g in range(1, NG):
            nc.sync.dma_start(wq_sb[g][:], wq4[:, g])
        for g in range(NG):
            nc.gpsimd.dma_start(wk_sb[g][:], wk4[:, g])
        for g in range(NG):
            nc.gpsimd.dma_start(wv_sb[g][:], wv4[:, g])

        wo_sb = const.tile([P, 2 * D], bf, tag="wo")
        bqk_sb = const.tile([P, 4], f32, tag="bqk")
        mask_sb = const.tile([P, 4 * QBLK], bf, tag="masks")
        ones_sb = const.tile([P, P], bf, tag="ones")
        nc.gpsimd.dma_start(bqk_sb[:], bqk[:])
        nc.gpsimd.dma_start(mask_sb[:], masks[:])
        nc.gpsimd.dma_start(ones_sb[:], ones[:])
        nc.gpsimd.dma_start(wo_sb[:], wo[:])

        # persistent activations
        q_all = persist.tile([P, 2 * T], bf, tag="q_all")      # roped qT
        k_all = persist.tile([P, 2 * T], bf, tag="k_all")      # roped kT
        v_all = persist.tile([P, 32 * 256], bf, tag="v_all")   # v natural
        at_all = persist.tile([P, 2 * T], bf, tag="at_all")    # attnT

        # ---------------- Phase 1: QKV projections + RoPE ----------------
        with tc.tile_pool(name="psum1", bufs=3, space="PSUM") as psum:
            for tcix in range(NCH):
                t0 = tcix * TCH
                if tcix == 0:
                    xt = xt0
                else:
                    xt = [xt_pool.tile([P, GA, TCH], bf, tag=f"xt{g}",
                                       name=f"xt{g}_{tcix}")
                          for g in range(NG)]
                    for g in range(NG):
                        nc.sync.dma_start(xt[g][:], xT4[:, g, :, t0:t0 + TCH])
                cosc = cs_pool.tile([P, 2, TCH], bf, tag="cos")
                nc.sync.dma_start(cosc[:], cosT3[:, :, t0:t0 + TCH])
                sinc = cs_pool.tile([P, 2, TCH], bf, tag="sin")
                nc.sync.dma_start(sinc[:], sinT3[:, :, t0:t0 + TCH])

                qraw = raw_pool.tile([P, 2, TCH], bf, tag="qraw")
                kraw = raw_pool.tile([P, 2, TCH], bf, tag="kraw")
                for (wt, rawt, bcol) in ((wq_sb, qraw, 0), (wk_sb, kraw, 2)):
                    for cb in range(2):
                        ps = psum.tile([P, TCH], f32, tag="qk")
                        for a in range(NB):
                            nc.tensor.matmul(
                                ps[:],
                                wt[a // GA][:, a % GA,
                                            cb * P:cb * P + P],
                                xt[a // GA][:, a % GA, :],
                                start=(a == 0), stop=(a == NB - 1),
                            )
                        nc.vector.tensor_scalar_add(
                            rawt[:, cb, :], ps[:],
                            bqk_sb[:, bcol + cb:bcol + cb + 1])
                # v: x-stationary, natural layout
                for tt in range(TCH // P):
                    ps = psum.tile([P, 256], f32, tag="v")
                    for a in range(NB):
                        nc.tensor.matmul(
                            ps[:],
                            xt[a // GA][:, a % GA, tt * P:(tt + 1) * P],
                            wv_sb[a // GA][:, a % GA, :],
                            start=(a == 0), stop=(a == NB - 1),
                        )
                    cidx = (tcix * (TCH // P) + tt) * 256
                    nc.scalar.activation(v_all[:, cidx:cidx + 256], ps[:],
                                         Act.Copy)

                # RoPE: rope_b = raw_b*cos_b + raw_{1-b}*sinT_b (sign-folded)
                for (rawt, dst) in ((qraw, q_all), (kraw, k_all)):
                    for cb in range(2):
                        tm = tmp_pool.tile([P, TCH], bf, tag="ropetmp")
                        nc.vector.tensor_mul(tm[:], rawt[:, 1 - cb, :],
                                             sinc[:, cb, :])
                        tm2 = tmp_pool.tile([P, TCH], bf, tag="ropetmp2")
                        nc.vector.tensor_mul(tm2[:], rawt[:, cb, :],
                                             cosc[:, cb, :])
                        nc.vector.tensor_add(
                            dst[:, cb * T + t0:cb * T + t0 + TCH],
                            tm[:], tm2[:])

        # ------- Phase 2+3: causal attention + output projection -------
        # scoresT blocks [kj=128, q=512]; exp on ScalarE over 2-bank pairs;
        # rowsum via ones-matmul accumulation; PV consumes expT directly.
        # The inner loop is software-pipelined (rs/pv trail sc/exp by one
        # pair) so PE never head-of-line blocks on ScalarE's exp. The
        # output projection for a (b, qj) token group is interleaved one
        # group late, once its at_all slices are long since written.
        def attn_group(psum, b, cb, qj):
            qs = cb * T + b * S + qj * QBLK
            nkb = 4 * qj + 4  # key blocks 0..nkb-1
            nquads = nkb // 4
            pv_ps = psum.tile([P, QBLK], f32, tag="pv", bufs=1,
                              name=f"pv_{b}{cb}{qj}")
            rs_ps = psum.tile([P, QBLK], f32, tag="rs", bufs=1,
                              name=f"rs_{b}{cb}{qj}")
            quad_buf = []

            def consume(ii, ex):
                for h in range(2):
                    i = 2 * ii + h
                    if i >= 4 * qj:  # diagonal: causal mask
                        dd = i - 4 * qj
                        nc.vector.tensor_mul(
                            ex[:, h, :], ex[:, h, :],
                            mask_sb[:, dd * QBLK:(dd + 1) * QBLK])
                    vix = (b * 16 + i) * 256 + cb * P
                    nc.tensor.matmul(pv_ps[:], v_all[:, vix:vix + P],
                                     ex[:, h, :],
                                     start=(i == 0), stop=(i == nkb - 1))
                # rowsum: pre-sum 4 exp blocks on DVE, one ones-matmul per quad
                quad_buf.append(ex)
                if len(quad_buf) == 2:
                    e0, e1 = quad_buf
                    quad_buf.clear()
                    qi = ii // 2
                    ea = tmp_pool.tile([P, QBLK], bf, tag="esA",
                                       bufs=2, name=f"esA_{b}{cb}{qj}_{qi}")
                    nc.vector.tensor_add(ea[:], e0[:, 0, :], e0[:, 1, :])
                    eb = tmp_pool.tile([P, QBLK], bf, tag="esB",
                                       bufs=2, name=f"esB_{b}{cb}{qj}_{qi}")
                    nc.vector.tensor_add(eb[:], e1[:, 0, :], e1[:, 1, :])
                    es = tmp_pool.tile([P, QBLK], bf, tag="esC",
                                       bufs=2, name=f"esC_{b}{cb}{qj}_{qi}")
                    nc.vector.tensor_add(es[:], ea[:], eb[:])
                    nc.tensor.matmul(rs_ps[:], ones_sb[:], es[:],
                                     start=(qi == 0), stop=(qi == nquads - 1))

            state = {"prev": None}

            def pair_step(ii, filler):
                sc_ps = psum.tile([P, 2, QBLK], f32, tag="sc",
                                  name=f"sc_{b}{cb}{qj}_{ii}")
                for h in range(2):
                    i = 2 * ii + h
                    ks = cb * T + b * S + i * P
                    nc.tensor.matmul(sc_ps[:, h, :], k_all[:, ks:ks + P],
                                     q_all[:, qs:qs + QBLK],
                                     start=True, stop=True)
                ex = exp_pool.tile([P, 2, QBLK], bf, tag="exp",
                                   name=f"ex_{b}{cb}{qj}_{ii}")
                nc.scalar.activation(ex[:], sc_ps[:], Act.Exp, scale=SCALE)
                # independent PE work lands here, between the exp issue and
                # the rs/pv matmuls that wait on it (PE executes in order)
                filler()
                if state["prev"] is not None:
                    consume(*state["prev"])
                state["prev"] = (ii, ex)

            def finish(filler):
                consume(*state["prev"])
                rec = rec_pool.tile([P, QBLK], f32, tag="rec",
                                    name=f"rec_{b}{cb}{qj}")
                nc.vector.reciprocal_approx_fast(rec[:], rs_ps[:])
                nc.vector.tensor_mul(at_all[:, qs:qs + QBLK], pv_ps[:], rec[:])
                filler()

            steps = [(lambda f, ii=ii: pair_step(ii, f))
                     for ii in range(nkb // 2)]
            steps.append(finish)
            return steps

        def out_units(psum, b, qj):
            # output projection for the 4 token chunks of (b, qj), split
            # into per-(token, dcol) units so they can fill PE bubbles
            # inside the next attention group's exp-chain.
            units = []

            def unit(tx, dc, orow_box):
                tt = (b * S + qj * QBLK) // P + tx
                if dc == 0:
                    orow_box.append(orow_pool.tile([P, D], bf, tag="orow",
                                                   name=f"orow_{tt}"))
                orow = orow_box[0]
                ps = psum.tile([P, 512], f32, tag="out",
                               name=f"out_{tt}_{dc}")
                for cb in range(2):
                    nc.tensor.matmul(
                        ps[:],
                        at_all[:, cb * T + tt * P:cb * T + (tt + 1) * P],
                        wo_sb[:, cb * D + dc * 512:cb * D + (dc + 1) * 512],
                        start=(cb == 0), stop=(cb == 1),
                    )
                # alternate eviction engine between ACT and DVE
                dst = orow[:, dc * 512:(dc + 1) * 512]
                if dc % 2 == 0:
                    nc.scalar.activation(dst, ps[:], Act.Copy)
                else:
                    nc.vector.tensor_copy(dst, ps[:])
                if dc == D // 512 - 1:
                    nc.sync.dma_start(out[tt * P:(tt + 1) * P, :], orow[:])

            for tx in range(QBLK // P):
                box = []
                for dc in range(D // 512):
                    units.append(lambda tx=tx, dc=dc, box=box: unit(tx, dc, box))
            return units

        with tc.tile_pool(name="psum2", bufs=2, space="PSUM") as psum:
            groups = [(b, qj) for b in range(B) for qj in range(NQ)]
            for gi, (b, qj) in enumerate(groups):
                steps = attn_group(psum, b, 0, qj) + attn_group(psum, b, 1, qj)
                outs = out_units(psum, *groups[gi - 1]) if gi >= 1 else []
                k = 0
                for si, st in enumerate(steps):
                    tgt = (si + 1) * len(outs) // len(steps)

                    def filler(tgt=tgt):
                        nonlocal k
                        while k < tgt:
                            outs[k]()
                            k += 1
                    st(filler)
            for u in out_units(psum, *groups[-1]):
                u()

    nc.compile()
    return nc


def _host_prep(x, cos, sin, Wq, bq, Wk, bk, Wv, bv, Wo, bo):
    """Build per-core input maps (numpy, bf16 on-device dtypes)."""
    def pblock(arr, nblk):
        # [nblk*128, F] -> [128, nblk*F] with col = a*F + f
        nb, f = nblk, arr.shape[1]
        return np.ascontiguousarray(
            arr.reshape(nb, P, f).transpose(1, 0, 2).reshape(P, nb * f))

    x2 = np.asarray(x, np.float32).reshape(T, D)
    xT_r = pblock(np.ascontiguousarray(x2.T), NB).astype(BF16)  # [128, 16*4096]

    cosn = np.asarray(cos, np.float32)
    sinn = np.asarray(sin, np.float32)
    Wqn = np.asarray(Wq, np.float32)
    Wkn = np.asarray(Wk, np.float32)
    Wvn = np.asarray(Wv, np.float32)
    Won = np.asarray(Wo, np.float32)
    bqn = np.asarray(bq, np.float32)
    bkn = np.asarray(bk, np.float32)

    # causal masks for the 4 diagonal offsets
    kj = np.arange(P)[:, None]
    qq = np.arange(QBLK)[None, :]
    mask4 = np.concatenate(
        [(qq - kj >= 128 * dd).astype(np.float32) for dd in range(4)], axis=1)

    common = {
        "xT": xT_r,
        "masks": mask4.astype(BF16),
        "ones": np.ones((P, P), BF16),
    }

    in_maps = []
    for m in range(N_CORES):
        cols = np.r_[128 * m:128 * m + 128, 1024 + 128 * m:1024 + 128 * m + 128]
        wq_s = pblock(Wqn[:, cols], NB).astype(BF16)
        wk_s = pblock(Wkn[:, cols], NB).astype(BF16)
        wv_s = pblock(Wvn[:, cols], NB).astype(BF16)
        wo_s = pblock(Won[cols, :], 2).astype(BF16)

        ct = np.tile(cosn[:, cols].T, (1, B))          # [256, 4096]
        st = np.tile(sinn[:, cols].T, (1, B)).copy()
        st[:128] *= -1.0                               # sign-fold block0
        cos_s = pblock(ct, 2).astype(BF16)
        sin_s = pblock(st, 2).astype(BF16)

        bqk_s = np.stack([bqn[cols[:128]], bqn[cols[128:]],
                          bkn[cols[:128]], bkn[cols[128:]]], axis=1)
        bqk_s = np.ascontiguousarray(bqk_s, np.float32)

        in_maps.append(dict(common, wq=wq_s, wk=wk_s, wv=wv_s, wo=wo_s,
                            cosT=cos_s, sinT=sin_s, bqk=bqk_s))
    return in_maps


def kernel(x, cos, sin, Wq, bq, Wk, bk, Wv, bv, Wo, bo):
    global LAST_RESULTS
    from concourse.bass_utils import run_bass_kernel_spmd

    if "nc" not in _CACHE:
        _CACHE["nc"] = _build_program()
    nc = _CACHE["nc"]

    in_maps = _host_prep(x, cos, sin, Wq, bq, Wk, bk, Wv, bv, Wo, bo)
    res = run_bass_kernel_spmd(nc, in_maps, core_ids=list(range(N_CORES)))
    LAST_RESULTS = res

    acc = np.zeros((T, D), np.float32)
    for r in res.results:
        acc += r["out"].astype(np.float32)
    # v-bias and output bias: attn rows sum to 1, so bv contributes bv @ Wo.
    acc += (np.asarray(bv, np.float32) @ np.asarray(Wo, np.float32)
            + np.asarray(bo, np.float32))[None, :]
    return acc.reshape(B, S, D)
